# revision 13
# baseline (speedup 1.0000x reference)
"""FPS (farthest point sampling) Trainium2 kernel, custom-DVE edition.

Problem: x (64, 65536, 3) fp32 -> y (64, 2048, 3): per cloud, iteratively
select the point maximizing min-distance-to-selected-set, starting at index 0
(exact argmax semantics incl. first-index tie-breaks).

Sharding: data-parallel over batch. 8 clouds per core; inside a core, 2
groups of 4 clouds processed as [128 partitions x 2048 free] planes
(cloud = 32 partitions). Per FPS iteration per group, FOUR fused custom
DVE passes (all fp32 bit-exact; DVE ALUs are exact IEEE fp32, unlike the
ACT spline Square), registered at runtime into dve_ops.OPS:
  u   = (xs-px)^2 + (ys-py)^2                       [FPS_SQ2ADD]
  v   = u + (zs-pz)^2                               [FPS_SQADD]
  md  = min(md, v); pm = rowmax(md)                 [FPS_MINRED, accum max]
  enc = pbase - argmax_first(md==pm)                [FPS_IDXSCAN: scan-eq
        counts elements before the first max; accum_init=pbase]
The winning point's coords feed the next iteration's squares as
per-partition scalars read straight from PSUM (ps_b).

Tail per iteration per group, fully off the critical DVE stream:
PE-transposes put pm (during IDXSCAN) and enc into one PSUM row; DVE does
the per-cloud (32-lane) winner reduce straight from PSUM:
gm=reduce_max, (pm>=gm)*enc via 2 TTs, wenc=reduce_max; PE transposes
wenc [1,4]->[4,1]; ACT computes rows = K_g - enc with int32 cast into the
row log rlog4 [4, M]; a 4-descriptor indirect DMA gathers the winners'
coords; a PE matmul with a block-ones [4,128] stationary broadcasts them
to all 128 partitions of PSUM ps_b (emitted late so the next group's
transposes are not queued behind it).

Emission is software-pipelined so each group's ~5us winner/gather/
broadcast chain hides under the other group's 4 distance passes:
  P12(1,t) B(0,t) P34(1,t) bcast(0,t) B(1,t) P12(0,t+1) P34(0,t+1) bcast(1,t)
Winner rows are written out once at the end; the final y gather happens
on the host (y = x[rows]). Ties are exact: scan-eq picks the first
in-partition index; across partitions max of enc = smallest global index.
"""
import sys
import types
import numpy as np

B, N, M = 64, 65536, 2048
NCORES = 8
BPC = B // NCORES          # clouds per core = 8
NGROUPS = 2
CPG = BPC // NGROUPS       # clouds per group = 4
PP = 128 // CPG            # partitions per cloud = 32
FD = N // PP               # free dim per partition = 2048
BIGK = float(1 << 24)
FLT_MAX = 3.4028235e38

_cached = {}


def _install_compat():
    """Environment workarounds: NTFF hook shim + 1-sync-wait-per-instruction
    splitter for this walrus build."""
    try:
        from antenv import axon_hooks  # noqa: F401
    except ImportError:
        try:
            from trn_agent_boot.trn_boot import _ntff_profile_via_ctypes
            _hook = _ntff_profile_via_ctypes('/opt/axon/libaxon_pjrt.so')
        except Exception:
            _hook = None
        _mod = types.ModuleType("antenv.axon_hooks")
        _mod.get_axon_ntff_profile_hook = lambda: _hook
        _mod.set_axon_ntff_profile_hook = lambda h: None
        sys.modules['antenv.axon_hooks'] = _mod

    import concourse.tile as tile_mod
    import concourse.mybir as mybir
    from bass_rust import ScopedClock
    import bass_rust as _br

    if getattr(tile_mod.TileContext, "_fps_patched", False):
        return
    tile_mod.TileContext._fps_patched = True

    _orig_lower = tile_mod.TileContext._lower_ordered_insts

    def _split_waits(self, ordered):
        sem_ids = {}
        try:
            for nm, h in self.sems.allocated().items():
                sem_ids[getattr(h, "name", nm)] = h.num
        except Exception:
            pass
        for bb_name, insts in ordered.items():
            out = []
            for inst in insts:
                si = inst.sync_info
                if type(inst).__name__ == "InstIncSwdgeSem":
                    # This walrus can't encode IncSwdgeSem (extended ISA).
                    # Replace with per-sem NOPs: one wait + one sem-inc each
                    # (mode 'sub' -> negative increments).
                    names = inst._sem_names
                    vals = inst._sem_values
                    mode = str(inst._mode)
                    sgn = -1 if "sub" in mode else 1
                    waits = {w.ant_name: w for w in (
                        list(si.on_wait) if si is not None else [])}
                    for nm, v in zip(names, vals):
                        upd = _br.SyncUpdate(
                            sync_type='semaphore', id=sem_ids[nm],
                            ant_name=nm, update_mode='sem-inc',
                            update_value=sgn * v, update_reg=None)
                        w = waits.pop(nm, None)
                        nop = mybir.InstNoOp(
                            name=self.nc.get_next_instruction_name(),
                            engine=inst.engine,
                            sync_info=mybir.SyncInfo(
                                on_wait=[w] if w is not None else [],
                                on_update=[upd]),
                            bass_nofuse=True,
                        )
                        out.append(nop)
                    for w in waits.values():
                        nop = mybir.InstNoOp(
                            name=self.nc.get_next_instruction_name(),
                            engine=inst.engine,
                            sync_info=mybir.SyncInfo(on_wait=[w], on_update=[]),
                            bass_nofuse=True,
                        )
                        out.append(nop)
                    continue
                if si is not None and len(si.on_wait) > 1:
                    waits = list(si.on_wait)
                    for w in waits[:-1]:
                        nop = mybir.InstNoOp(
                            name=self.nc.get_next_instruction_name(),
                            engine=inst.engine,
                            sync_info=mybir.SyncInfo(on_wait=[w], on_update=[]),
                            bass_nofuse=True,
                        )
                        out.append(nop)
                    si.on_wait = waits[-1:]
                    inst.sync_info = si
                out.append(inst)
            insts[:] = out
        return _orig_lower(self, ordered)

    tile_mod.TileContext._lower_ordered_insts = _split_waits

    def _patched_drain_and_barrier(self, tick_clock, wait_clock):
        probe = self.nc.sync.nop(nofuse=True)
        wait_clock.add_sem_waits(
            probe.ins, ScopedClock({None: tick_clock.global_clock})
        )
        si = probe.ins.sync_info
        waits = list(si.on_wait)
        if len(waits) > 1:
            si.on_wait = waits[:1]
            probe.ins.sync_info = si
            for w in waits[1:]:
                extra = self.nc.sync.nop(nofuse=True)
                extra.ins.sync_info = _br.SyncInfo(on_wait=[w], on_update=[])
        self.nc.sync.drain()
        self.nc.all_engine_barrier()
        assert self.sems is not None
        popped = self.nc._tile_sem_poison_stack.pop()
        assert popped is self._sem_poison
        # NOTE: skip gpsimd dma_reset/sem_clear (range sem_clear emits an
        # InstISA this walrus rejects); only do the free-list bookkeeping.
        sems = list(self.sems.allocated().values())
        if sems:
            sem_nums = [getattr(s_, "num", s_) for s_ in sems]
            self.nc._state.prepend_free_semaphores(sem_nums)
            for poison_set in self.nc._tile_sem_poison_stack:
                poison_set.update(sem_nums)
        self.nc.all_engine_barrier()

    tile_mod.TileContext._drain_and_barrier = _patched_drain_and_barrier


_ops_cache = {}


def _fps_ops():
    """Register (once) and return the custom DVE ops used by the kernel."""
    if _ops_cache:
        return _ops_cache
    from concourse import dve_ops as DO
    from concourse.dve_spec import (
        Spec, Src0, Src1, C0, C1, AluOp, eq, sq, minn, scan, lower, One,
    )
    from concourse.dve_uop import DveOpSpec

    def make_op(name, spec, subdim=False, perf=False):
        if name in DO._SUB_OPCODE_FOR_NAME:
            return next(o for o in DO.OPS if o.name == name)
        shas = {}
        for ver in ("v3", "v4"):
            s = DveOpSpec(name=name, opcode=0, uops=lower(spec, ver=ver),
                          rd1_en=DO.has_src1(spec))
            shas[ver] = s.sha(ver)
        op = DO.DveOp(name, spec, subdim, shas,
                      perf_en={"v3": perf, "v4": perf} if perf else {})
        DO.OPS.append(op)
        DO.CUSTOM_DVE_SPECS[name] = spec
        DO._SUB_OPCODE_FOR_NAME[name] = DO._CUSTOM_DVE_ROW_BASE + len(DO.OPS) - 1
        assert DO._SUB_OPCODE_FOR_NAME[name] < 0x20
        return op

    _ops_cache["SQ2ADD"] = make_op("FPS_SQ2ADD", Spec(
        body=sq(Src0 - C0) + sq(Src1 - C1),
        reference=lambda in0, in1, s0, s1, imm2: (
            (in0 - s0) * (in0 - s0) + (in1 - s1) * (in1 - s1)
        ).astype(np.float32),
    ))

    _ops_cache["SQADD"] = make_op("FPS_SQADD", Spec(
        body=Src0 + sq(Src1 - C0),
        reference=lambda in0, in1, s0, s1, imm2: (
            in0 + (in1 - s0) * (in1 - s0)
        ).astype(np.float32),
    ))

    def _minred_ref(in0, in1, s0, s1, imm2):
        out = np.minimum(in0, in1).astype(np.float32)
        acc = np.maximum(
            s0, out.reshape(out.shape[0], -1).max(axis=-1, keepdims=True))
        return out, acc

    _ops_cache["MINRED"] = make_op("FPS_MINRED", Spec(
        body=minn(Src0, Src1),
        accum=AluOp.MAX,
        accum_init=C0,
        reference=_minred_ref,
    ))

    def _idx_ref(in0, in1, s0, s1, imm2):
        m = np.maximum.accumulate((in0 == s0).astype(np.float32), axis=-1)
        out = m - 1.0
        acc = s1 + out.reshape(out.shape[0], -1).sum(axis=-1, keepdims=True)
        return out, acc

    _ops_cache["IDXSCAN"] = make_op("FPS_IDXSCAN", Spec(
        body=scan(AluOp.MAX, eq(Src0, C0)) - One,
        accum=AluOp.ADD,
        accum_init=C1,
        reference=_idx_ref,
    ))
    return _ops_cache


def _build(n_iters=M):
    import concourse.bass as bass
    import concourse.mybir as mybir
    from concourse.tile import TileContext
    from concourse.bass import IndirectOffsetOnAxis

    ops = _fps_ops()
    SQ2ADD, SQADD = ops["SQ2ADD"], ops["SQADD"]
    MINRED, IDXSCAN = ops["MINRED"], ops["IDXSCAN"]

    fp = mybir.dt.float32
    i32 = mybir.dt.int32
    nc = bass.Bass("TRN2", target_bir_lowering=False, debug=False)

    x_d = nc.dram_tensor("x", [BPC * N, 3], fp, kind="ExternalInput")
    rows_d = nc.dram_tensor("rows_out", [NGROUPS * CPG, M], i32,
                            kind="ExternalOutput")
    ident_d = nc.dram_tensor("ident", [128, 128], fp, kind="ExternalInput")
    pbase_d = nc.dram_tensor("pbase", [128, 1], fp, kind="ExternalInput")
    bones_d = nc.dram_tensor("bones", [CPG, 128], fp, kind="ExternalInput")
    rows0_d = nc.dram_tensor("rows0", [NGROUPS * CPG, 1], i32,
                             kind="ExternalInput")

    with TileContext(nc) as tc:
        import contextlib
        with contextlib.ExitStack() as ctx:
            cpool = ctx.enter_context(tc.tile_pool(name="consts", bufs=1))
            ident = cpool.tile([128, 128], fp, tag="ident")
            nc.sync.dma_start(ident[:, :], ident_d[:, :])
            pbase = cpool.tile([128, 1], fp, tag="pbase")
            nc.sync.dma_start(pbase[:, :], pbase_d[:, :])
            bones = cpool.tile([CPG, 128], fp, tag="bones")
            nc.sync.dma_start(bones[:, :], bones_d[:, :])
            junk = cpool.tile([128, FD], fp, tag="junk")

            G = []  # per-group state
            for g in range(NGROUPS):
                gp = ctx.enter_context(tc.tile_pool(name=f"g{g}", bufs=1))
                pg = ctx.enter_context(
                    tc.tile_pool(name=f"p{g}", bufs=1, space="PSUM"))
                st = {}
                for nm in ("xs", "ys", "zs", "md", "u", "v"):
                    st[nm] = gp.tile([128, FD], fp, tag=nm, name=f"{nm}_{g}")
                # double-buffered small tiles (break cross-engine WARs)
                for nm, shape, dt_ in (
                        ("pm1", [128, 1], fp), ("encp", [128, 1], fp),
                        ("gm4", [1, CPG], fp),
                        ("wB", [1, 128], fp), ("wE", [1, 128], fp),
                        ("wenc", [1, CPG], fp),
                        ("pbc4", [CPG, 3], fp)):
                    st[nm] = [gp.tile(shape, dt_, tag=f"{nm}{k}",
                                      name=f"{nm}{k}_{g}") for k in (0, 1)]
                st["rlog4"] = gp.tile([CPG, M], i32, tag="rlog4",
                                      name=f"rlog4_{g}")
                # two PSUM banks per parity: bank A holds the pm row +
                # rows-T + coord broadcast; bank B holds the enc row alone
                # so the winner reduce's pm wait is not coupled to the enc
                # transpose / broadcast matmuls in the PE queue.
                psbA = [pg.tile([128, 512], fp, tag=f"psbA{k}",
                                name=f"psbA{k}_{g}") for k in (0, 1)]
                psbB = [pg.tile([128, 512], fp, tag=f"psbB{k}",
                                name=f"psbB{k}_{g}") for k in (0, 1)]
                st["ps_tp"] = [b[0:1, 0:128] for b in psbA]
                st["ps_te"] = [b[0:1, 0:128] for b in psbB]
                st["ps_r"] = [b[0:CPG, 256:257] for b in psbA]
                st["ps_b"] = [b[:, 260:263] for b in psbA]
                G.append(st)

                # load x contiguously, then split into coordinate planes
                xall = gp.tile([128, FD * 3], fp, tag="xall",
                               name=f"xall_{g}")
                xv2 = x_d.ap().rearrange("(p f) c -> p (f c)", f=FD)
                base = g * CPG * PP
                for sl in range(0, 128, 16):
                    nc.sync.dma_start(
                        xall[sl:sl + 16, :],
                        xv2[base + sl:base + sl + 16, :])
                x3 = xall[:, :].rearrange("p (f c) -> p f c", c=3)
                for nm, c in (("xs", 0), ("ys", 1), ("zs", 2)):
                    nc.vector.tensor_copy(st[nm][:, :], x3[:, :, c])
                nc.vector.memset(st["md"][:, :], FLT_MAX)

                # initial point = index 0 of each cloud (rows0), coords
                # gathered (4 descriptors) then PE-broadcast to 128 parts
                nc.sync.dma_start(
                    st["rlog4"][:, 0:1],
                    rows0_d[g * CPG:(g + 1) * CPG, :])
                nc.gpsimd.indirect_dma_start(
                    out=st["pbc4"][0][:, :], out_offset=None,
                    in_=x_d[:, :],
                    in_offset=IndirectOffsetOnAxis(
                        ap=st["rlog4"][:, 0:1], axis=0),
                )
                nc.tensor.matmul(st["ps_b"][0][:, :], bones[:, :],
                                 st["pbc4"][0][:, :])

            from concourse.tile import add_dep_helper
            order_prev = {"i": None}

            def chain(inst):
                # force DVE issue order across groups/phases
                if order_prev["i"] is not None:
                    add_dep_helper(inst.ins, order_prev["i"].ins, sync=False,
                                   reason="DVE issue order")
                order_prev["i"] = inst

            def emit_B_head_pm(g, t):
                """pm -> [1,128] PSUM row: runs during IDXSCAN (PE)."""
                st = G[g]
                cv = t % 2
                nc.tensor.transpose(st["ps_tp"][cv][0:1, :],
                                    st["pm1"][cv][:, :], ident[:, :])

            def emit_B_head_enc(g, t):
                st = G[g]
                cv = t % 2
                nc.tensor.transpose(st["ps_te"][cv][0:1, :],
                                    st["encp"][cv][:, :], ident[:, :])

            def emit_B_dve_a(g, t):
                """DVE smalls 1/2: per-cloud max + winner mask."""
                st = G[g]
                cv = t % 2
                gm4 = st["gm4"][cv]
                wB = st["wB"][cv]
                pmv = st["ps_tp"][cv][0:1, :].rearrange(
                    "o (c p) -> o c p", p=PP)
                chain(nc.vector.reduce_max(
                    out=gm4[:, :], in_=pmv, axis=mybir.AxisListType.X))
                chain(nc.vector.tensor_tensor(
                    out=wB[0:1, :].rearrange("o (c p) -> o c p", p=PP),
                    in0=pmv,
                    in1=gm4[0:1, :].rearrange("o (c z) -> o c z", z=1)
                    .broadcast_to([1, CPG, PP]),
                    op=mybir.AluOpType.is_ge))

            def emit_B_dve_b(g, t):
                """DVE smalls 2/2: winner enc + rows/log/gather chain."""
                st = G[g]
                cv = t % 2
                wB, wE = st["wB"][cv], st["wE"][cv]
                wenc = st["wenc"][cv]
                pbc4 = st["pbc4"][cv]
                ps_r, ps_b = st["ps_r"][cv], st["ps_b"][cv]
                env = st["ps_te"][cv][0:1, :].rearrange(
                    "o (c p) -> o c p", p=PP)
                chain(nc.vector.tensor_tensor(
                    out=wE[0:1, :].rearrange("o (c p) -> o c p", p=PP),
                    in0=wB[0:1, :].rearrange("o (c p) -> o c p", p=PP),
                    in1=env, op=mybir.AluOpType.mult))
                chain(nc.vector.reduce_max(
                    out=wenc[:, :],
                    in_=wE[0:1, :].rearrange("o (c p) -> o c p", p=PP),
                    axis=mybir.AxisListType.X))
                # winner enc to 4 partitions (PE transpose)
                nc.tensor.transpose(ps_r[:, :], wenc[:, :], ident[0:1, 0:1])
                # rows = K_g - enc, cast int32, log: on the scalar engine
                nc.scalar.activation(
                    st["rlog4"][:, t:t + 1], ps_r[:, :],
                    mybir.ActivationFunctionType.Copy,
                    bias=BIGK + g * CPG * N, scale=-1.0)
                # gather winner coords (4 descriptors)
                nc.gpsimd.indirect_dma_start(
                    out=pbc4[:, :], out_offset=None,
                    in_=x_d[:, :],
                    in_offset=IndirectOffsetOnAxis(
                        ap=st["rlog4"][:, t:t + 1], axis=0),
                )

            def emit_B_bcast(g, t):
                # coord broadcast via PE: emitted late so the other group's
                # pm/enc transposes aren't queued behind this 1.1us matmul
                st = G[g]
                cv = t % 2
                nc.tensor.matmul(st["ps_b"][cv][:, :], bones[:, :],
                                 st["pbc4"][cv][:, :])

            # Software-pipelined emission. DVE stream per iteration:
            #   P12(g0) P34(g0) | P12(g1) | B(g0) | P34(g1) | ...next iter
            # so each group's B-phase (winner select -> gather -> coord
            # broadcast, ~4us of PE/ACT/DMA latency after its last DVE op)
            # hides under the other group's distance passes.
            def emit_P1(g, t):
                st = G[g]
                psb = st["ps_b"][(t - 1) % 2]
                chain(nc.vector._custom_dve(
                    SQ2ADD, out=st["u"][:, :], in0=st["xs"][:, :],
                    in1=st["ys"][:, :], s0=psb[:, 0:1], s1=psb[:, 1:2]))

            def emit_P2(g, t):
                st = G[g]
                psb = st["ps_b"][(t - 1) % 2]
                chain(nc.vector._custom_dve(
                    SQADD, out=st["v"][:, :], in0=st["u"][:, :],
                    in1=st["zs"][:, :], s0=psb[:, 2:3]))

            def emit_P12(g, t):
                emit_P1(g, t)
                emit_P2(g, t)

            def emit_P34(g, t):
                st = G[g]
                cv = t % 2
                chain(nc.vector._custom_dve(
                    MINRED, out=st["md"][:, :], in0=st["md"][:, :],
                    in1=st["v"][:, :], s0=-FLT_MAX,
                    accum_out=st["pm1"][cv][:, :]))
                emit_B_head_pm(g, t)
                chain(nc.vector._custom_dve(
                    IDXSCAN, out=junk[:, :], in0=st["md"][:, :],
                    s0=st["pm1"][cv][:, 0:1], s1=pbase[:, 0:1],
                    accum_out=st["encp"][cv][:, :]))
                emit_B_head_enc(g, t)

            emit_P12(0, 1)
            emit_P34(0, 1)
            for t in range(1, n_iters):
                emit_P1(1, t)
                emit_B_dve_a(0, t)
                emit_P2(1, t)
                emit_B_dve_b(0, t)
                emit_P34(1, t)
                emit_B_bcast(0, t)
                if t + 1 < n_iters:
                    emit_P1(0, t + 1)
                    emit_B_dve_a(1, t)
                    emit_P2(0, t + 1)
                    emit_B_dve_b(1, t)
                    emit_P34(0, t + 1)
                    emit_B_bcast(1, t)
                else:
                    emit_B_dve_a(1, t)
                    emit_B_dve_b(1, t)

            for g in range(NGROUPS):
                nc.sync.dma_start(
                    rows_d[g * CPG:(g + 1) * CPG, :],
                    G[g]["rlog4"][:, :])

    from concourse.library_overlay import lower_extended_insts
    lower_extended_insts(nc)
    return nc


def _host_consts():
    ident = np.eye(128, dtype=np.float32)
    pbase = (BIGK - np.arange(128, dtype=np.float64) * FD).astype(
        np.float32).reshape(128, 1)
    bones = np.zeros((CPG, 128), dtype=np.float32)
    for c in range(CPG):
        bones[c, c * PP:(c + 1) * PP] = 1.0
    rows0 = (np.arange(BPC, dtype=np.int32) * N).reshape(NGROUPS * CPG, 1)
    return ident, pbase, bones, rows0


def run_device(x, n_iters=M, trace=False):
    """Run the device part; returns (rows arrays per core, exec_time_ns)."""
    _install_compat()
    from concourse.bass_utils import run_bass_kernel_spmd

    key = ("nc", n_iters)
    if key not in _cached:
        _cached[key] = _build(n_iters)
    nc = _cached[key]

    ident, pbase, bones, rows0 = _host_consts()
    x = np.ascontiguousarray(x, dtype=np.float32)
    in_maps = []
    for core in range(NCORES):
        shard = x[core * BPC:(core + 1) * BPC].reshape(BPC * N, 3)
        in_maps.append({
            "x": shard, "ident": ident, "pbase": pbase, "bones": bones,
            "rows0": rows0,
        })
    res = run_bass_kernel_spmd(nc, in_maps, core_ids=list(range(NCORES)),
                               trace=trace)
    _cached["last_res"] = res
    rows = [res.results[i]["rows_out"] for i in range(NCORES)]
    return rows, res.exec_time_ns


def decode_rows(rows_list, n_iters=M):
    """rows arrays (per core [BPC, M] global shard rows) -> (B, n_iters)."""
    idx = np.zeros((B, n_iters), dtype=np.int64)
    for core in range(NCORES):
        rows = rows_list[core].astype(np.int64)[:, :n_iters]  # [BPC, n_iters]
        idx[core * BPC:(core + 1) * BPC] = rows % N
    return idx


def kernel(x: np.ndarray) -> np.ndarray:
    x = np.ascontiguousarray(x, dtype=np.float32)
    rows_list, _ = run_device(x)
    idx = decode_rows(rows_list)
    y = np.take_along_axis(x, idx[:, :, None].astype(np.int64), axis=1)
    return y.astype(np.float32)


# revision 14
# speedup vs baseline: 1.0053x; 1.0053x over previous
"""FPS (farthest point sampling) Trainium2 kernel, custom-DVE edition.

Problem: x (64, 65536, 3) fp32 -> y (64, 2048, 3): per cloud, iteratively
select the point maximizing min-distance-to-selected-set, starting at index 0
(exact argmax semantics incl. first-index tie-breaks).

Sharding: data-parallel over batch. 8 clouds per core; inside a core, 2
groups of 4 clouds processed as [128 partitions x 2048 free] planes
(cloud = 32 partitions). Per FPS iteration per group, FOUR fused custom
DVE passes (all fp32 bit-exact; DVE ALUs are exact IEEE fp32, unlike the
ACT spline Square), registered at runtime into dve_ops.OPS:
  u   = (xs-px)^2 + (ys-py)^2                       [FPS_SQ2ADD]
  v   = u + (zs-pz)^2                               [FPS_SQADD]
  md  = min(md, v); pm = rowmax(md)                 [FPS_MINRED, accum max]
  enc = pbase - argmax_first(md==pm)                [FPS_IDXSCAN: scan-eq
        counts elements before the first max; accum_init=pbase]
The winning point's coords feed the next iteration's squares as
per-partition scalars read straight from PSUM (ps_b).

Tail per iteration per group, fully off the critical DVE stream:
PE-transposes put pm (during IDXSCAN) and enc into one PSUM row; DVE does
the per-cloud (32-lane) winner reduce straight from PSUM:
gm=reduce_max, (pm>=gm)*enc via 2 TTs, wenc=reduce_max; PE transposes
wenc [1,4]->[4,1]; ACT computes rows = K_g - enc with int32 cast into the
row log rlog4 [4, M]; a 4-descriptor indirect DMA gathers the winners'
coords; a PE matmul with a block-ones [4,128] stationary broadcasts them
to all 128 partitions of PSUM ps_b (emitted late so the next group's
transposes are not queued behind it).

Emission is software-pipelined so each group's ~5us winner/gather/
broadcast chain hides under the other group's 4 distance passes:
  P12(1,t) B(0,t) P34(1,t) bcast(0,t) B(1,t) P12(0,t+1) P34(0,t+1) bcast(1,t)
Winner rows are written out once at the end; the final y gather happens
on the host (y = x[rows]). Ties are exact: scan-eq picks the first
in-partition index; across partitions max of enc = smallest global index.
"""
import sys
import types
import numpy as np

B, N, M = 64, 65536, 2048
NCORES = 8
BPC = B // NCORES          # clouds per core = 8
NGROUPS = 2
CPG = BPC // NGROUPS       # clouds per group = 4
PP = 128 // CPG            # partitions per cloud = 32
FD = N // PP               # free dim per partition = 2048
BIGK = float(1 << 24)
FLT_MAX = 3.4028235e38

_cached = {}


def _install_compat():
    """Environment workarounds: NTFF hook shim + 1-sync-wait-per-instruction
    splitter for this walrus build."""
    try:
        from antenv import axon_hooks  # noqa: F401
    except ImportError:
        try:
            from trn_agent_boot.trn_boot import _ntff_profile_via_ctypes
            _hook = _ntff_profile_via_ctypes('/opt/axon/libaxon_pjrt.so')
        except Exception:
            _hook = None
        _mod = types.ModuleType("antenv.axon_hooks")
        _mod.get_axon_ntff_profile_hook = lambda: _hook
        _mod.set_axon_ntff_profile_hook = lambda h: None
        sys.modules['antenv.axon_hooks'] = _mod

    import concourse.tile as tile_mod
    import concourse.mybir as mybir
    from bass_rust import ScopedClock
    import bass_rust as _br

    if getattr(tile_mod.TileContext, "_fps_patched", False):
        return
    tile_mod.TileContext._fps_patched = True

    _orig_lower = tile_mod.TileContext._lower_ordered_insts

    def _split_waits(self, ordered):
        sem_ids = {}
        try:
            for nm, h in self.sems.allocated().items():
                sem_ids[getattr(h, "name", nm)] = h.num
        except Exception:
            pass
        for bb_name, insts in ordered.items():
            out = []
            for inst in insts:
                si = inst.sync_info
                if type(inst).__name__ == "InstIncSwdgeSem":
                    # This walrus can't encode IncSwdgeSem (extended ISA).
                    # Replace with per-sem NOPs: one wait + one sem-inc each
                    # (mode 'sub' -> negative increments).
                    names = inst._sem_names
                    vals = inst._sem_values
                    mode = str(inst._mode)
                    sgn = -1 if "sub" in mode else 1
                    waits = {w.ant_name: w for w in (
                        list(si.on_wait) if si is not None else [])}
                    for nm, v in zip(names, vals):
                        upd = _br.SyncUpdate(
                            sync_type='semaphore', id=sem_ids[nm],
                            ant_name=nm, update_mode='sem-inc',
                            update_value=sgn * v, update_reg=None)
                        w = waits.pop(nm, None)
                        nop = mybir.InstNoOp(
                            name=self.nc.get_next_instruction_name(),
                            engine=inst.engine,
                            sync_info=mybir.SyncInfo(
                                on_wait=[w] if w is not None else [],
                                on_update=[upd]),
                            bass_nofuse=True,
                        )
                        out.append(nop)
                    for w in waits.values():
                        nop = mybir.InstNoOp(
                            name=self.nc.get_next_instruction_name(),
                            engine=inst.engine,
                            sync_info=mybir.SyncInfo(on_wait=[w], on_update=[]),
                            bass_nofuse=True,
                        )
                        out.append(nop)
                    continue
                if si is not None and len(si.on_wait) > 1:
                    waits = list(si.on_wait)
                    for w in waits[:-1]:
                        nop = mybir.InstNoOp(
                            name=self.nc.get_next_instruction_name(),
                            engine=inst.engine,
                            sync_info=mybir.SyncInfo(on_wait=[w], on_update=[]),
                            bass_nofuse=True,
                        )
                        out.append(nop)
                    si.on_wait = waits[-1:]
                    inst.sync_info = si
                out.append(inst)
            insts[:] = out
        return _orig_lower(self, ordered)

    tile_mod.TileContext._lower_ordered_insts = _split_waits

    def _patched_drain_and_barrier(self, tick_clock, wait_clock):
        probe = self.nc.sync.nop(nofuse=True)
        wait_clock.add_sem_waits(
            probe.ins, ScopedClock({None: tick_clock.global_clock})
        )
        si = probe.ins.sync_info
        waits = list(si.on_wait)
        if len(waits) > 1:
            si.on_wait = waits[:1]
            probe.ins.sync_info = si
            for w in waits[1:]:
                extra = self.nc.sync.nop(nofuse=True)
                extra.ins.sync_info = _br.SyncInfo(on_wait=[w], on_update=[])
        self.nc.sync.drain()
        self.nc.all_engine_barrier()
        assert self.sems is not None
        popped = self.nc._tile_sem_poison_stack.pop()
        assert popped is self._sem_poison
        # NOTE: skip gpsimd dma_reset/sem_clear (range sem_clear emits an
        # InstISA this walrus rejects); only do the free-list bookkeeping.
        sems = list(self.sems.allocated().values())
        if sems:
            sem_nums = [getattr(s_, "num", s_) for s_ in sems]
            self.nc._state.prepend_free_semaphores(sem_nums)
            for poison_set in self.nc._tile_sem_poison_stack:
                poison_set.update(sem_nums)
        self.nc.all_engine_barrier()

    tile_mod.TileContext._drain_and_barrier = _patched_drain_and_barrier


_ops_cache = {}


def _fps_ops():
    """Register (once) and return the custom DVE ops used by the kernel."""
    if _ops_cache:
        return _ops_cache
    from concourse import dve_ops as DO
    from concourse.dve_spec import (
        Spec, Src0, Src1, C0, C1, AluOp, eq, sq, minn, scan, lower, One,
    )
    from concourse.dve_uop import DveOpSpec

    def make_op(name, spec, subdim=False, perf=False):
        if name in DO._SUB_OPCODE_FOR_NAME:
            return next(o for o in DO.OPS if o.name == name)
        shas = {}
        for ver in ("v3", "v4"):
            s = DveOpSpec(name=name, opcode=0, uops=lower(spec, ver=ver),
                          rd1_en=DO.has_src1(spec))
            shas[ver] = s.sha(ver)
        op = DO.DveOp(name, spec, subdim, shas,
                      perf_en={"v3": perf, "v4": perf} if perf else {})
        DO.OPS.append(op)
        DO.CUSTOM_DVE_SPECS[name] = spec
        DO._SUB_OPCODE_FOR_NAME[name] = DO._CUSTOM_DVE_ROW_BASE + len(DO.OPS) - 1
        assert DO._SUB_OPCODE_FOR_NAME[name] < 0x20
        return op

    _ops_cache["SQ2ADD"] = make_op("FPS_SQ2ADD", Spec(
        body=sq(Src0 - C0) + sq(Src1 - C1),
        reference=lambda in0, in1, s0, s1, imm2: (
            (in0 - s0) * (in0 - s0) + (in1 - s1) * (in1 - s1)
        ).astype(np.float32),
    ))

    _ops_cache["SQADD"] = make_op("FPS_SQADD", Spec(
        body=Src0 + sq(Src1 - C0),
        reference=lambda in0, in1, s0, s1, imm2: (
            in0 + (in1 - s0) * (in1 - s0)
        ).astype(np.float32),
    ))

    def _minred_ref(in0, in1, s0, s1, imm2):
        out = np.minimum(in0, in1).astype(np.float32)
        acc = np.maximum(
            s0, out.reshape(out.shape[0], -1).max(axis=-1, keepdims=True))
        return out, acc

    _ops_cache["MINRED"] = make_op("FPS_MINRED", Spec(
        body=minn(Src0, Src1),
        accum=AluOp.MAX,
        accum_init=C0,
        reference=_minred_ref,
    ))

    def _idx_ref(in0, in1, s0, s1, imm2):
        m = np.maximum.accumulate((in0 == s0).astype(np.float32), axis=-1)
        out = m - 1.0
        acc = s1 + out.reshape(out.shape[0], -1).sum(axis=-1, keepdims=True)
        return out, acc

    _ops_cache["IDXSCAN"] = make_op("FPS_IDXSCAN", Spec(
        body=scan(AluOp.MAX, eq(Src0, C0)) - One,
        accum=AluOp.ADD,
        accum_init=C1,
        reference=_idx_ref,
    ))
    return _ops_cache


def _build(n_iters=M):
    import concourse.bass as bass
    import concourse.mybir as mybir
    from concourse.tile import TileContext
    from concourse.bass import IndirectOffsetOnAxis

    ops = _fps_ops()
    SQ2ADD, SQADD = ops["SQ2ADD"], ops["SQADD"]
    MINRED, IDXSCAN = ops["MINRED"], ops["IDXSCAN"]

    fp = mybir.dt.float32
    i32 = mybir.dt.int32
    nc = bass.Bass("TRN2", target_bir_lowering=False, debug=False)

    x_d = nc.dram_tensor("x", [BPC * N, 3], fp, kind="ExternalInput")
    rows_d = nc.dram_tensor("rows_out", [NGROUPS * CPG, M], i32,
                            kind="ExternalOutput")
    ident_d = nc.dram_tensor("ident", [128, 128], fp, kind="ExternalInput")
    pbase_d = nc.dram_tensor("pbase", [128, 1], fp, kind="ExternalInput")
    bones_d = nc.dram_tensor("bones", [CPG, 128], fp, kind="ExternalInput")
    rows0_d = nc.dram_tensor("rows0", [NGROUPS * CPG, 1], i32,
                             kind="ExternalInput")

    with TileContext(nc) as tc:
        import contextlib
        with contextlib.ExitStack() as ctx:
            cpool = ctx.enter_context(tc.tile_pool(name="consts", bufs=1))
            ident = cpool.tile([128, 128], fp, tag="ident")
            nc.sync.dma_start(ident[:, :], ident_d[:, :])
            pbase = cpool.tile([128, 1], fp, tag="pbase")
            nc.sync.dma_start(pbase[:, :], pbase_d[:, :])
            bones = cpool.tile([CPG, 128], fp, tag="bones")
            nc.sync.dma_start(bones[:, :], bones_d[:, :])
            junk = cpool.tile([128, FD], fp, tag="junk")

            G = []  # per-group state
            for g in range(NGROUPS):
                gp = ctx.enter_context(tc.tile_pool(name=f"g{g}", bufs=1))
                pg = ctx.enter_context(
                    tc.tile_pool(name=f"p{g}", bufs=1, space="PSUM"))
                st = {}
                for nm in ("xs", "ys", "zs", "md", "u", "v"):
                    st[nm] = gp.tile([128, FD], fp, tag=nm, name=f"{nm}_{g}")
                # double-buffered small tiles (break cross-engine WARs)
                for nm, shape, dt_ in (
                        ("pm1", [128, 1], fp), ("encp", [128, 1], fp),
                        ("gm4", [1, CPG], fp),
                        ("wB", [1, 128], fp), ("wE", [1, 128], fp),
                        ("wenc", [1, CPG], fp),
                        ("pbc4", [CPG, 3], fp)):
                    st[nm] = [gp.tile(shape, dt_, tag=f"{nm}{k}",
                                      name=f"{nm}{k}_{g}") for k in (0, 1)]
                st["rlog4"] = gp.tile([CPG, M], i32, tag="rlog4",
                                      name=f"rlog4_{g}")
                # two PSUM banks per parity: bank A holds the pm row +
                # rows-T + coord broadcast; bank B holds the enc row alone
                # so the winner reduce's pm wait is not coupled to the enc
                # transpose / broadcast matmuls in the PE queue.
                psbA = [pg.tile([128, 512], fp, tag=f"psbA{k}",
                                name=f"psbA{k}_{g}") for k in (0, 1)]
                psbB = [pg.tile([128, 512], fp, tag=f"psbB{k}",
                                name=f"psbB{k}_{g}") for k in (0, 1)]
                st["ps_tp"] = [b[0:1, 0:128] for b in psbA]
                st["ps_te"] = [b[0:1, 0:128] for b in psbB]
                st["ps_r"] = [b[0:CPG, 256:257] for b in psbA]
                st["ps_b"] = [b[:, 260:263] for b in psbA]
                G.append(st)

                # load x contiguously, then split into coordinate planes
                xall = gp.tile([128, FD * 3], fp, tag="xall",
                               name=f"xall_{g}")
                xv2 = x_d.ap().rearrange("(p f) c -> p (f c)", f=FD)
                base = g * CPG * PP
                for sl in range(0, 128, 16):
                    nc.sync.dma_start(
                        xall[sl:sl + 16, :],
                        xv2[base + sl:base + sl + 16, :])
                x3 = xall[:, :].rearrange("p (f c) -> p f c", c=3)
                for nm, c in (("xs", 0), ("ys", 1), ("zs", 2)):
                    nc.vector.tensor_copy(st[nm][:, :], x3[:, :, c])
                nc.vector.memset(st["md"][:, :], FLT_MAX)

                # initial point = index 0 of each cloud (rows0), coords
                # gathered (4 descriptors) then PE-broadcast to 128 parts
                nc.sync.dma_start(
                    st["rlog4"][:, 0:1],
                    rows0_d[g * CPG:(g + 1) * CPG, :])
                nc.gpsimd.indirect_dma_start(
                    out=st["pbc4"][0][:, :], out_offset=None,
                    in_=x_d[:, :],
                    in_offset=IndirectOffsetOnAxis(
                        ap=st["rlog4"][:, 0:1], axis=0),
                )
                nc.tensor.matmul(st["ps_b"][0][:, :], bones[:, :],
                                 st["pbc4"][0][:, :])

            from concourse.tile import add_dep_helper
            order_prev = {"i": None}

            def chain(inst):
                # force DVE issue order across groups/phases
                if order_prev["i"] is not None:
                    add_dep_helper(inst.ins, order_prev["i"].ins, sync=False,
                                   reason="DVE issue order")
                order_prev["i"] = inst

            def emit_B_head_pm(g, t):
                """pm -> [1,128] PSUM row: runs during IDXSCAN (PE)."""
                st = G[g]
                cv = t % 2
                nc.tensor.transpose(st["ps_tp"][cv][0:1, :],
                                    st["pm1"][cv][:, :], ident[:, :])

            def emit_B_head_enc(g, t):
                st = G[g]
                cv = t % 2
                nc.tensor.transpose(st["ps_te"][cv][0:1, :],
                                    st["encp"][cv][:, :], ident[:, :])

            def emit_B_dve_a(g, t):
                """DVE smalls 1/2: per-cloud max + winner mask."""
                st = G[g]
                cv = t % 2
                gm4 = st["gm4"][cv]
                wB = st["wB"][cv]
                pmv = st["ps_tp"][cv][0:1, :].rearrange(
                    "o (c p) -> o c p", p=PP)
                chain(nc.vector.reduce_max(
                    out=gm4[:, :], in_=pmv, axis=mybir.AxisListType.X))
                chain(nc.vector.tensor_tensor(
                    out=wB[0:1, :].rearrange("o (c p) -> o c p", p=PP),
                    in0=pmv,
                    in1=gm4[0:1, :].rearrange("o (c z) -> o c z", z=1)
                    .broadcast_to([1, CPG, PP]),
                    op=mybir.AluOpType.is_ge))

            def emit_B_dve_b(g, t):
                """DVE smalls 2/2: winner enc + rows/log/gather chain."""
                st = G[g]
                cv = t % 2
                wB, wE = st["wB"][cv], st["wE"][cv]
                wenc = st["wenc"][cv]
                pbc4 = st["pbc4"][cv]
                ps_r, ps_b = st["ps_r"][cv], st["ps_b"][cv]
                env = st["ps_te"][cv][0:1, :].rearrange(
                    "o (c p) -> o c p", p=PP)
                chain(nc.vector.tensor_tensor(
                    out=wE[0:1, :].rearrange("o (c p) -> o c p", p=PP),
                    in0=wB[0:1, :].rearrange("o (c p) -> o c p", p=PP),
                    in1=env, op=mybir.AluOpType.mult))
                chain(nc.vector.reduce_max(
                    out=wenc[:, :],
                    in_=wE[0:1, :].rearrange("o (c p) -> o c p", p=PP),
                    axis=mybir.AxisListType.X))
                # winner enc to 4 partitions (PE transpose)
                nc.tensor.transpose(ps_r[:, :], wenc[:, :], ident[0:1, 0:1])
                # rows = K_g - enc, cast int32, log: on the scalar engine
                nc.scalar.activation(
                    st["rlog4"][:, t:t + 1], ps_r[:, :],
                    mybir.ActivationFunctionType.Copy,
                    bias=BIGK + g * CPG * N, scale=-1.0)
                # gather winner coords (4 descriptors)
                nc.gpsimd.indirect_dma_start(
                    out=pbc4[:, :], out_offset=None,
                    in_=x_d[:, :],
                    in_offset=IndirectOffsetOnAxis(
                        ap=st["rlog4"][:, t:t + 1], axis=0),
                )

            def emit_B_bcast(g, t):
                # coord broadcast via PE: emitted late so the other group's
                # pm/enc transposes aren't queued behind this 1.1us matmul
                st = G[g]
                cv = t % 2
                nc.tensor.matmul(st["ps_b"][cv][:, :], bones[:, :],
                                 st["pbc4"][cv][:, :])

            # Software-pipelined emission. DVE stream per iteration:
            #   P12(g0) P34(g0) | P12(g1) | B(g0) | P34(g1) | ...next iter
            # so each group's B-phase (winner select -> gather -> coord
            # broadcast, ~4us of PE/ACT/DMA latency after its last DVE op)
            # hides under the other group's distance passes.
            def emit_P1(g, t):
                st = G[g]
                psb = st["ps_b"][(t - 1) % 2]
                chain(nc.vector._custom_dve(
                    SQ2ADD, out=st["u"][:, :], in0=st["xs"][:, :],
                    in1=st["ys"][:, :], s0=psb[:, 0:1], s1=psb[:, 1:2]))

            def emit_P2(g, t):
                st = G[g]
                psb = st["ps_b"][(t - 1) % 2]
                chain(nc.vector._custom_dve(
                    SQADD, out=st["v"][:, :], in0=st["u"][:, :],
                    in1=st["zs"][:, :], s0=psb[:, 2:3]))

            def emit_P12(g, t):
                emit_P1(g, t)
                emit_P2(g, t)

            def emit_P34(g, t):
                st = G[g]
                cv = t % 2
                chain(nc.vector._custom_dve(
                    MINRED, out=st["md"][:, :], in0=st["md"][:, :],
                    in1=st["v"][:, :], s0=-FLT_MAX,
                    accum_out=st["pm1"][cv][:, :]))
                emit_B_head_pm(g, t)
                chain(nc.vector._custom_dve(
                    IDXSCAN, out=junk[:, :], in0=st["md"][:, :],
                    s0=st["pm1"][cv][:, 0:1], s1=pbase[:, 0:1],
                    accum_out=st["encp"][cv][:, :]))
                emit_B_head_enc(g, t)

            emit_P12(0, 1)
            emit_P34(0, 1)
            for t in range(1, n_iters):
                emit_P12(1, t)
                emit_B_dve_a(0, t)
                emit_B_dve_b(0, t)
                emit_P34(1, t)
                emit_B_bcast(0, t)
                emit_B_dve_a(1, t)
                emit_B_dve_b(1, t)
                if t + 1 < n_iters:
                    emit_P12(0, t + 1)
                    emit_P34(0, t + 1)
                emit_B_bcast(1, t)

            for g in range(NGROUPS):
                nc.sync.dma_start(
                    rows_d[g * CPG:(g + 1) * CPG, :],
                    G[g]["rlog4"][:, :])

    from concourse.library_overlay import lower_extended_insts
    lower_extended_insts(nc)
    return nc


def _host_consts():
    ident = np.eye(128, dtype=np.float32)
    pbase = (BIGK - np.arange(128, dtype=np.float64) * FD).astype(
        np.float32).reshape(128, 1)
    bones = np.zeros((CPG, 128), dtype=np.float32)
    for c in range(CPG):
        bones[c, c * PP:(c + 1) * PP] = 1.0
    rows0 = (np.arange(BPC, dtype=np.int32) * N).reshape(NGROUPS * CPG, 1)
    return ident, pbase, bones, rows0


def run_device(x, n_iters=M, trace=False):
    """Run the device part; returns (rows arrays per core, exec_time_ns)."""
    _install_compat()
    from concourse.bass_utils import run_bass_kernel_spmd

    key = ("nc", n_iters)
    if key not in _cached:
        _cached[key] = _build(n_iters)
    nc = _cached[key]

    ident, pbase, bones, rows0 = _host_consts()
    x = np.ascontiguousarray(x, dtype=np.float32)
    in_maps = []
    for core in range(NCORES):
        shard = x[core * BPC:(core + 1) * BPC].reshape(BPC * N, 3)
        in_maps.append({
            "x": shard, "ident": ident, "pbase": pbase, "bones": bones,
            "rows0": rows0,
        })
    res = run_bass_kernel_spmd(nc, in_maps, core_ids=list(range(NCORES)),
                               trace=trace)
    _cached["last_res"] = res
    rows = [res.results[i]["rows_out"] for i in range(NCORES)]
    return rows, res.exec_time_ns


def decode_rows(rows_list, n_iters=M):
    """rows arrays (per core [BPC, M] global shard rows) -> (B, n_iters)."""
    idx = np.zeros((B, n_iters), dtype=np.int64)
    for core in range(NCORES):
        rows = rows_list[core].astype(np.int64)[:, :n_iters]  # [BPC, n_iters]
        idx[core * BPC:(core + 1) * BPC] = rows % N
    return idx


def kernel(x: np.ndarray) -> np.ndarray:
    x = np.ascontiguousarray(x, dtype=np.float32)
    rows_list, _ = run_device(x)
    idx = decode_rows(rows_list)
    y = np.take_along_axis(x, idx[:, :, None].astype(np.int64), axis=1)
    return y.astype(np.float32)
